# revision 12
# baseline (speedup 1.0000x reference)
"""Trainium2 Bass kernel for nn_LogReg_30193620091430 — v3.

Per core (data-parallel over 8 cores, 1250 graphs / 125k nodes each):

  DMA layout: seq is loaded in 1 MiB blocks of 1024 consecutive nodes with
  partition p holding 8 consecutive rows (nodes 1024*B + 8p .. +8p+7), so
  every DMA descriptor is one contiguous 8 KiB line -> 128 descriptors per
  MiB (8x fewer than a node-per-partition layout) and near-line-rate HBM.
  Loads alternate between the two HWDGE queues (sync+scalar).

  stage 1: segment-sum via f32r matmuls: stationary = 0/1 pattern
  [128, 64-graph cols] for (block%25, j), moving = seq slice [128, 256].
  PSUM accumulates 64-graph groups ([64, 256]); blocks straddling a group
  boundary are split at the (always partition-aligned) boundary.
  Patterns ship as uint8 and are cast to f32r once by the vector engine.

  stage 2 per group: PSUM->SBUF copy (vector), PE transpose of both
  128-feature halves, FC with host-pretransposed W + bias via ones-row
  matmul, PReLU as one scalar_tensor_tensor max(a*x, x) (valid for a <= 1).
  Output stores ride the gpsimd SWDGE queue so they never block seq loads.
"""
import numpy as np

NUM_GRAPHS = 10000
NODES_PER_GRAPH = 100
FT_IN = 256
NB_CLASSES = 128
N_CORES = 8

G_CORE = NUM_GRAPHS // N_CORES            # 1250 graphs per core
N_CORE = G_CORE * NODES_PER_GRAPH         # 125000 nodes per core
NBLK = N_CORE // 1024                     # 122 full 1024-node blocks
TAILROWS = N_CORE - NBLK * 1024           # 72-node tail (9 partitions x 8)
GB = 64                                   # graphs per PSUM group
NGROUPS = (G_CORE + GB - 1) // GB         # 20 groups (19 full + one of 34)

_CACHE = {}


def _contributions(q):
    """(block, p0, p1, bm_eff) spans of 64-graph group q. Group boundaries
    inside a block are partition-aligned (offsets 256/512/768 -> p* of
    32/64/96).  matmul base partitions may only be 0/32/64, so the p0=96
    span uses base 64 with pattern slot 25: the bm=18 pattern with rows
    64..95 zeroed (those nodes belong to the previous group)."""
    S, ph = q // 4, q % 4
    base = 25 * S
    if ph == 0:
        return ([(base + b, 0, 128, b) for b in range(6)]
                + [(base + 6, 0, 32, 6)])
    if ph == 1:
        # base-32 APs may span at most 32 partitions -> split at 64
        return ([(base + 6, 32, 64, 6), (base + 6, 64, 128, 6)]
                + [(base + b, 0, 128, b) for b in range(7, 12)]
                + [(base + 12, 0, 64, 12)])
    if ph == 2:
        return ([(base + 12, 64, 128, 12)]
                + [(base + b, 0, 128, b) for b in range(13, 18)]
                + [(base + 18, 0, 96, 18)])
    return ([(base + 18, 64, 128, 25)]
            + [(base + b, 0, 128, b) for b in range(19, 25)])


def _build_module():
    import concourse.bacc as bacc
    import concourse.mybir as mybir
    from concourse.tile import TileContext

    F32 = mybir.dt.float32
    F32R = mybir.dt.float32r
    U8 = mybir.dt.uint8
    F = FT_IN
    C = NB_CLASSES

    # pat[p, ((bm*8 + j)*64 + gl)] = 1 iff ((1024*bm + 8p + j)//100) % 64 == gl
    # slot bm=25: the bm=18 pattern with rows p<96 zeroed (for the p0=64
    # base-partition workaround of the p*=96 group boundary).
    p_ = np.arange(128)[:, None, None]
    bm_ = np.arange(25)[None, :, None]
    j_ = np.arange(8)[None, None, :]
    node = 1024 * bm_ + 8 * p_ + j_
    gl = (node // 100) % GB
    pat = np.zeros((128, 26, 8, GB), dtype=np.uint8)
    P, BM, J = np.broadcast_arrays(p_, bm_, j_)
    pat[P.ravel(), BM.ravel(), J.ravel(), gl.ravel()] = 1
    pat[96:, 25] = pat[96:, 18]
    pat = pat.reshape(128, 26 * 8 * GB)
    PCOLS = pat.shape[1]                   # 13312

    nc = bacc.Bacc(None, target_bir_lowering=False)
    seq = nc.dram_tensor("seq", [N_CORE, F], F32, kind="ExternalInput")
    wt = nc.dram_tensor("wt", [128, 2 * C], F32, kind="ExternalInput")
    b_in = nc.dram_tensor("b", [1, C], F32, kind="ExternalInput")
    ones_in = nc.dram_tensor("ones", [1, GB], F32, kind="ExternalInput")
    acol_in = nc.dram_tensor("a_col", [GB, 1], F32, kind="ExternalInput")
    id_in = nc.dram_tensor("ident", [GB, GB], F32, kind="ExternalInput")
    out = nc.dram_tensor("out", [G_CORE, C], F32, kind="ExternalOutput")

    patt_d = nc.inline_tensor(pat, name="patt_u8")

    with TileContext(nc) as tc:
        with (
            tc.tile_pool(name="const", bufs=1) as cpool,
            tc.tile_pool(name="seqp", bufs=12) as seqp,
            tc.tile_pool(name="s2", bufs=3) as s2,
            tc.tile_pool(name="ps1", bufs=4, space="PSUM") as ps1,
            tc.tile_pool(name="pst", bufs=2, space="PSUM") as pst,
            tc.tile_pool(name="ps2", bufs=2, space="PSUM") as ps2,
        ):
            patt_u8 = cpool.tile([128, PCOLS], U8)
            nc.gpsimd.dma_start(patt_u8[:, :], patt_d[:, :])
            patt = cpool.tile([128, PCOLS], F32R)
            nc.vector.tensor_copy(patt[:, :], patt_u8[:, :])

            wt_sb = cpool.tile([128, 2 * C], F32R)
            nc.gpsimd.dma_start(wt_sb[:, :], wt[:, :].bitcast(F32R))
            b_sb = cpool.tile([1, C], F32R)
            nc.gpsimd.dma_start(b_sb[:, :], b_in[:, :].bitcast(F32R))
            ones_t = cpool.tile([1, GB], F32R)
            nc.gpsimd.dma_start(ones_t[:, :], ones_in[:, :].bitcast(F32R))
            a_col = cpool.tile([GB, 1], F32)
            nc.gpsimd.dma_start(a_col[:, :], acol_in[:, :])
            ident_t = cpool.tile([GB, GB], F32)
            nc.gpsimd.dma_start(ident_t[:, :], id_in[:, :])

            # seq blocks: partition p <- rows 1024*B + 8p .. +7 (8KB lines)
            seq_blk = seq[:NBLK * 1024, :].rearrange(
                "(w p j) f -> p w j f", p=128, j=8)
            blk_tiles = []
            for b in range(NBLK):
                eng = nc.sync if b % 2 == 0 else nc.scalar
                sq = seqp.tile([128, 8 * F], F32R)
                eng.dma_start(
                    sq[:, :].rearrange("p (j f) -> p j f", j=8),
                    seq_blk[:, b, :, :].bitcast(F32R),
                )
                blk_tiles.append(sq)
            # tail: 72 rows = 9 partitions x 8 consecutive rows
            sq_tail = seqp.tile([9, 8 * F], F32R, tag="tail", bufs=1)
            nc.scalar.dma_start(
                sq_tail[:, :].rearrange("p (j f) -> p j f", j=8),
                seq[NBLK * 1024:, :].rearrange(
                    "(p j) f -> p j f", p=9).bitcast(F32R),
            )

            for q in range(NGROUPS):
                ng = min(GB, G_CORE - GB * q)
                conts = []
                for (blk, p0, p1, bm) in _contributions(q):
                    if blk > NBLK:
                        continue
                    if blk == NBLK:
                        if TAILROWS == 0 or p0 >= 9:
                            continue
                        p1 = min(p1, 9)
                    conts.append((blk, p0, p1, bm))

                pooled_ps = ps1.tile([GB, 512], F32)
                n_mm = len(conts) * 8
                i_mm = 0
                for (blk, p0, p1, bm) in conts:
                    sq = blk_tiles[blk] if blk < NBLK else sq_tail
                    for j in range(8):
                        pc = (bm * 8 + j) * GB
                        nc.tensor.matmul(
                            pooled_ps[:, :F],
                            patt[p0:p1, pc:pc + GB],
                            sq[p0:p1, j * F:(j + 1) * F],
                            start=(i_mm == 0), stop=(i_mm == n_mm - 1),
                        )
                        i_mm += 1

                # stage 2
                pooled_sb = s2.tile([GB, F], F32)
                nc.vector.tensor_copy(pooled_sb[:ng, :], pooled_ps[:ng, :F])
                pt_sb = s2.tile([128, 2 * GB], F32R)
                for h in range(2):
                    tp = pst.tile([128, 512], F32, tag="tp")
                    nc.tensor.transpose(
                        tp[:, :ng], pooled_sb[:ng, 128 * h:128 * (h + 1)],
                        ident_t[:ng, :ng])
                    nc.vector.tensor_copy(
                        pt_sb[:, GB * h:GB * h + ng], tp[:, :ng])

                ret_ps = ps2.tile([GB, 512], F32, tag="ret")
                nc.tensor.matmul(ret_ps[:ng, :C], ones_t[:1, :ng],
                                 b_sb[:1, :], start=True, stop=False)
                for h in range(2):
                    nc.tensor.matmul(
                        ret_ps[:ng, :C],
                        pt_sb[:, GB * h:GB * h + ng],
                        wt_sb[:, C * h:C * (h + 1)],
                        start=False, stop=(h == 1),
                    )
                ret_sb = s2.tile([GB, C], F32)
                nc.vector.tensor_copy(ret_sb[:ng, :], ret_ps[:ng, :C])
                out_sb = s2.tile([GB, C], F32)
                nc.vector.scalar_tensor_tensor(
                    out_sb[:ng, :], ret_sb[:ng, :], a_col[:ng, 0:1],
                    ret_sb[:ng, :],
                    op0=mybir.AluOpType.mult, op1=mybir.AluOpType.max,
                )
                nc.gpsimd.dma_start(out[GB * q:GB * q + ng, :],
                                    out_sb[:ng, :])

    nc.finalize()
    return nc


def prepare_in_maps(seq, graph_len, W, b, prelu_a):
    seq = np.ascontiguousarray(np.asarray(seq, dtype=np.float32))
    W = np.asarray(W, dtype=np.float32)
    # wt[p, 128h + c] = W[c, 128h + p]  (pretransposed halves of W)
    wt = np.empty((128, 2 * NB_CLASSES), dtype=np.float32)
    for h in range(2):
        wt[:, NB_CLASSES * h:NB_CLASSES * (h + 1)] = \
            W[:, 128 * h:128 * (h + 1)].T
    wt = np.ascontiguousarray(wt)
    b2 = np.ascontiguousarray(np.asarray(b, dtype=np.float32)
                              .reshape(1, NB_CLASSES))
    ones = np.ones((1, GB), dtype=np.float32)
    a_col = np.full((GB, 1), np.float32(np.asarray(prelu_a)),
                    dtype=np.float32)
    ident = np.eye(GB, dtype=np.float32)

    shards = seq.reshape(N_CORES, N_CORE, FT_IN)
    return [
        {"seq": shards[i], "wt": wt, "b": b2, "ones": ones,
         "a_col": a_col, "ident": ident}
        for i in range(N_CORES)
    ]


def kernel(seq, graph_len, W, b, prelu_a):
    from concourse.bass_utils import run_bass_kernel_spmd

    if "nc" not in _CACHE:
        _CACHE["nc"] = _build_module()
    nc = _CACHE["nc"]

    in_maps = prepare_in_maps(seq, graph_len, W, b, prelu_a)
    res = run_bass_kernel_spmd(nc, in_maps, core_ids=list(range(N_CORES)))
    return np.concatenate([r["out"] for r in res.results], axis=0)
